# revision 1
# baseline (speedup 1.0000x reference)
"""TRN2 Bass kernel for nn_Aggregator (GNN message passing + bi-interaction).

Computes, for graph with N=100000 nodes, E=800000 edges, D=128:
    msgs = entity_embed[src] * att                  (per-edge message)
    N_h  = segment_sum(msgs, dst)                   (scatter-add to nodes)
    out  = LRelu((node+N_h)@W1+b1) + LRelu((node*N_h)@W2+b2)

Strategy (8 NeuronCores, SPMD, no collectives):
  * Edges are bucketed by dst//12500 -> owning core; each core computes the
    full output rows for its 12500-node partition.
  * Within a core, edges are grouped into 128-node dst windows (98 windows),
    each padded to C=9 chunks of 128 edges (pads carry idx=0 and a zero
    column in S so they contribute nothing).
  * Per-edge embeddings are fetched with dma_gather (Q7 ucode) on FOUR SWDGE
    queues round-robin -- descriptor generation is the gather bottleneck and
    the queues select disjoint Q7 core pairs (~3.3x).  One gather call is
    capped at 1024 indices, so each window issues two calls (640+512).
    int16 gather indices only address 32768 rows, so the node table is
    compacted per (core, third-of-windows) into <=32768 distinct rows.
  * Segment-sum is a matmul: per 128-edge chunk, a host-precomputed
    selection matrix S[e, j] = (j == dst_local[e]) * att[e] rides in as an
    input (pure layout of att/dst), and N_h^T[d, win] += msgs[e,d]^T @ S
    accumulates in PSUM.  N=128 matmuls run at ~264 ns vs ~462 ns for N=64
    on this silicon, which sets the window width.
  * Everything downstream stays transposed [dim, node]: x1=nodeT+N_hT,
    x2=nodeT*N_hT (DVE), out1^T via lhsT=W1 as stored (PE), bias+LeakyReLU
    on the Scalar engine (bias is per-partition in the [od, node] layout),
    final add on DVE.  The kernel emits out^T in 128-node tiles; the host
    transposes once at the end.
"""
import sys

sys.path.insert(0, "/opt/trn_rl_repo")

import numpy as np

N_NODES = 100000
N_EDGES = 800000
D = 128
NCORES = 8
NPC = N_NODES // NCORES          # 12500 nodes per core
W = 128                          # dst window width (matmul N dim)
NWIN = (NPC + W - 1) // W        # 98 windows (= tiles) per core
C = 9                            # chunks (of 128 edges) per window
SLOTW = C * 128                  # 1152 edge slots per window
NPC_PAD = NWIN * 128             # padded node count per core (12544)
NCH = NWIN * C                   # 882 chunks per core
NSLOT = NCH * 128                # 112896 edge slots per core
TBL = 32768                      # padded gather-table rows (int16 limit)
THIRD_WINS = (33, 33, 32)        # window split -> 3 gather tables per core
NQ = 4                           # SWDGE queues (parallel gather desc-gen)

_BUILD_CACHE = {}


def _third_of_window(w):
    if w < THIRD_WINS[0]:
        return 0
    if w < THIRD_WINS[0] + THIRD_WINS[1]:
        return 1
    return 2


def _build(c_chunks=C):
    """Build + bacc-compile the SPMD Bass program (shape-static)."""
    key = (W, c_chunks, NQ, TBL)
    if key in _BUILD_CACHE:
        return _BUILD_CACHE[key]

    from contextlib import ExitStack
    import concourse.tile as tile
    from concourse import bacc, mybir

    f32 = mybir.dt.float32
    CC = c_chunks
    SLOTW_ = CC * 128
    NCH_ = NWIN * CC
    NSLOT_ = NCH_ * 128
    # chunk-aligned <=1024-idx gather call split for one window
    split = []
    left = CC
    while left > 0:
        n = min(8, left) if left != 9 else 5
        split.append(n)
        left -= n
    nc = bacc.Bacc("TRN2", target_bir_lowering=False, debug=False,
                   num_devices=NCORES, num_swdge_queues=NQ)

    tables = [nc.dram_tensor(f"table{t}", [TBL, D], f32, kind="ExternalInput").ap()
              for t in range(3)]
    idx_all = nc.dram_tensor("idx_all", [128, NSLOT_ // 16], mybir.dt.int16,
                             kind="ExternalInput").ap()
    s_mat = nc.dram_tensor("s_mat", [NWIN, 128, CC, W], f32,
                           kind="ExternalInput").ap()
    embedT = nc.dram_tensor("embedT", [NWIN, 128, 128], f32,
                            kind="ExternalInput").ap()
    w1 = nc.dram_tensor("w1", [D, D], f32, kind="ExternalInput").ap()
    w2 = nc.dram_tensor("w2", [D, D], f32, kind="ExternalInput").ap()
    b1 = nc.dram_tensor("b1", [D, 1], f32, kind="ExternalInput").ap()
    b2 = nc.dram_tensor("b2", [D, 1], f32, kind="ExternalInput").ap()
    outT = nc.dram_tensor("outT", [NWIN, 128, 128], f32,
                          kind="ExternalOutput").ap()

    with tile.TileContext(nc) as tc, ExitStack() as ctx:
        const = ctx.enter_context(tc.tile_pool(name="const", bufs=1))
        msgp = ctx.enter_context(tc.tile_pool(name="msg", bufs=10))
        sp = ctx.enter_context(tc.tile_pool(name="sp", bufs=6))
        ntp = ctx.enter_context(tc.tile_pool(name="ntp", bufs=3))
        xp = ctx.enter_context(tc.tile_pool(name="xp", bufs=4))
        rp = ctx.enter_context(tc.tile_pool(name="rp", bufs=4))
        op = ctx.enter_context(tc.tile_pool(name="op", bufs=3))
        psnh = ctx.enter_context(tc.tile_pool(name="psnh", bufs=4, space="PSUM"))
        psout = ctx.enter_context(tc.tile_pool(name="psout", bufs=2, space="PSUM"))

        idx_sb = const.tile([128, NSLOT_ // 16], mybir.dt.int16)
        nc.sync.dma_start(idx_sb[:], idx_all)
        w1_sb = const.tile([D, D], f32)
        nc.sync.dma_start(w1_sb[:], w1)
        w2_sb = const.tile([D, D], f32)
        nc.sync.dma_start(w2_sb[:], w2)
        b1_sb = const.tile([D, 1], f32)
        nc.sync.dma_start(b1_sb[:], b1)
        b2_sb = const.tile([D, 1], f32)
        nc.sync.dma_start(b2_sb[:], b2)

        # per-window gather: two calls (chunk-aligned) round-robin on queues
        msg_tiles = []
        qi = 0
        for w_i in range(NWIN):
            t = _third_of_window(w_i)
            m = msgp.tile([128, CC, D], f32, tag="msg")
            c0 = 0
            for nch in split:
                nidx = nch * 128
                off16 = (w_i * SLOTW_ + c0 * 128) // 16
                nc.gpsimd.dma_gather(
                    out_ap=m[:, c0 : c0 + nch, :],
                    in_ap=tables[t],
                    idxs_ap=idx_sb[:, off16 : off16 + nidx // 16],
                    num_idxs=nidx,
                    num_idxs_reg=nidx,
                    elem_size=D,
                    queue_num=qi % NQ,
                )
                qi += 1
                c0 += nch
            msg_tiles.append(m)

        lrelu = mybir.ActivationFunctionType.Lrelu
        pend = []            # (t, x1, x2) of recent windows, finals deferred

        def emit_finals(p):
            t_p, x1, x2 = p
            o1 = psout.tile([128, 128], f32, tag="o1")
            nc.tensor.matmul(out=o1[:], lhsT=w1_sb[:], rhs=x1[:],
                             start=True, stop=True)
            o2 = psout.tile([128, 128], f32, tag="o2")
            nc.tensor.matmul(out=o2[:], lhsT=w2_sb[:], rhs=x2[:],
                             start=True, stop=True)
            r1 = rp.tile([128, 128], f32, tag="r1")
            nc.scalar.activation(out=r1[:], in_=o1[:], func=lrelu,
                                 bias=b1_sb[:], scale=1.0, alpha=0.01)
            r2 = rp.tile([128, 128], f32, tag="r2")
            nc.scalar.activation(out=r2[:], in_=o2[:], func=lrelu,
                                 bias=b2_sb[:], scale=1.0, alpha=0.01)
            ot = op.tile([128, 128], f32, tag="ot")
            nc.vector.tensor_tensor(out=ot[:], in0=r1[:], in1=r2[:],
                                    op=mybir.AluOpType.add)
            nc.sync.dma_start(outT[t_p], ot[:])

        # windows processed in groups of three with their accumulation
        # chains interleaved: the PE is in-order, so when one window's gather
        # call hasn't landed yet the sibling windows' matmuls keep the array
        # busy (and the HAM clock warm); three chains cover ~5us of stall
        for g0 in range(0, NWIN, 3):
            grp = list(range(g0, min(g0 + 3, NWIN)))
            nhs = []
            sss = []
            for t in grp:
                nh_g = psnh.tile([128, 128], f32, tag="nh")
                s_g = sp.tile([128, CC, W], f32, tag="S")
                nc.sync.dma_start(s_g[:], s_mat[t])
                nhs.append(nh_g)
                sss.append(s_g)
            for cc in range(CC):
                for gi, t in enumerate(grp):
                    nc.tensor.matmul(
                        out=nhs[gi][:], lhsT=msg_tiles[t][:, cc, :],
                        rhs=sss[gi][:, cc, :],
                        start=(cc == 0), stop=(cc == CC - 1),
                    )
            for t, nh in zip(grp, nhs):
                nt = ntp.tile([128, 128], f32, tag="nt")
                nc.sync.dma_start(nt[:], embedT[t])
                x1 = xp.tile([128, 128], f32, tag="x1")
                nc.vector.tensor_tensor(out=x1[:], in0=nt[:], in1=nh[:],
                                        op=mybir.AluOpType.add)
                x2 = xp.tile([128, 128], f32, tag="x2")
                nc.vector.tensor_tensor(out=x2[:], in0=nt[:], in1=nh[:],
                                        op=mybir.AluOpType.mult)
                # finals of an earlier window issue here, after this pair's
                # chunk matmuls: they depend on DVE results only ready now
                pend.append((t, x1, x2))
                if len(pend) > 1:
                    emit_finals(pend.pop(0))
        for p in pend:
            emit_finals(p)

    nc.compile()
    _BUILD_CACHE[key] = nc
    return nc


def _prep_core(c, src, dst, att_flat, entity_embed, c_chunks=C):
    """Host-side slotting for one core. Returns the per-core input map."""
    SLOTW_ = c_chunks * 128
    NSLOT_ = NWIN * SLOTW_
    mask = (dst >= c * NPC) & (dst < (c + 1) * NPC)
    e_src = src[mask].astype(np.int64)
    e_att = att_flat[mask].astype(np.float32)
    ld = (dst[mask] - c * NPC).astype(np.int64)
    win = ld // W

    order = np.argsort(win, kind="stable")
    e_src, e_att, ld, win = e_src[order], e_att[order], ld[order], win[order]

    counts = np.bincount(win, minlength=NWIN)
    if counts.max() > SLOTW_:
        raise ValueError(f"window overflow: {counts.max()} edges > {SLOTW_}")
    cum = np.concatenate(([0], np.cumsum(counts)))[:-1]
    rank = np.arange(len(win)) - cum[win]
    slot = win * SLOTW_ + rank                       # global stream position

    att_slot = np.zeros(NSLOT_, np.float32)
    dstl_slot = np.zeros(NSLOT_, np.int64)
    src_slot = np.zeros(NSLOT_, np.int64)
    real = np.zeros(NSLOT_, bool)
    att_slot[slot] = e_att
    dstl_slot[slot] = ld - win * W
    src_slot[slot] = e_src
    real[slot] = True

    # compact gather tables per third of windows
    tables = []
    idx_local = np.zeros(NSLOT, np.int64)
    w0 = 0
    for nw in THIRD_WINS:
        s0, s1 = w0 * SLOTW_, (w0 + nw) * SLOTW_
        seg = src_slot[s0:s1]
        uniq, inv = np.unique(seg, return_inverse=True)
        if len(uniq) > TBL:
            raise ValueError(f"third table overflow: {len(uniq)} > {TBL}")
        tb = np.zeros((TBL, D), np.float32)
        tb[: len(uniq)] = entity_embed[uniq]
        tables.append(tb)
        idx_local[s0:s1] = inv
        w0 += nw

    # wrap-16 layout: idx position i -> [i%16, i//16], replicated to 128 rows
    idxw = idx_local.astype(np.int16).reshape(NSLOT_ // 16, 16).T
    idx_all = np.tile(idxw, (8, 1))

    # host-built selection matrices: S[w, p, c, k] = (k==dstl)*att of the
    # edge in slot (window w, chunk c, partition p); zero rows for pads
    s_mat = np.zeros((NSLOT_, W), np.float32)
    s_mat[np.arange(NSLOT_)[real], dstl_slot[real]] = att_slot[real]
    s_mat = s_mat.reshape(NWIN, c_chunks, 128, W).transpose(0, 2, 1, 3)
    s_mat = np.ascontiguousarray(s_mat)

    ep = np.zeros((NPC_PAD, D), np.float32)
    ep[:NPC] = entity_embed[c * NPC : (c + 1) * NPC]
    embedT = np.ascontiguousarray(
        ep.reshape(NWIN, 128, D).transpose(0, 2, 1))

    return dict(
        table0=tables[0], table1=tables[1], table2=tables[2],
        idx_all=idx_all, s_mat=s_mat, embedT=embedT,
    )


def kernel(entity_embed, att, W1, b1, W2, b2, src, dst):
    from concourse.bass_utils import run_bass_kernel_spmd

    entity_embed = np.ascontiguousarray(np.asarray(entity_embed, dtype=np.float32))
    att_flat = np.asarray(att, dtype=np.float32).reshape(-1)
    W1 = np.asarray(W1, dtype=np.float32)
    W2 = np.asarray(W2, dtype=np.float32)
    b1c = np.asarray(b1, dtype=np.float32).reshape(D, 1)
    b2c = np.asarray(b2, dtype=np.float32).reshape(D, 1)
    src = np.asarray(src).astype(np.int64)
    dst = np.asarray(dst).astype(np.int64)

    shared = dict(w1=W1, w2=W2, b1=b1c, b2=b2c)

    # chunks per window: C by default, bumped if any window is denser
    ld_all = dst % NPC
    win_id = (dst // NPC) * NWIN + ld_all // W
    max_edges = np.bincount(win_id, minlength=NCORES * NWIN).max()
    c_chunks = max(C, int(-(-int(max_edges) // 128)))

    in_maps = []
    for c in range(NCORES):
        m = _prep_core(c, src, dst, att_flat, entity_embed, c_chunks)
        m.update(shared)
        in_maps.append(m)

    nc = _build(c_chunks)
    res = run_bass_kernel_spmd(nc, in_maps, core_ids=list(range(NCORES)))

    out = np.empty((N_NODES, D), np.float32)
    for c in range(NCORES):
        o = res.results[c]["outT"]                   # [NWIN, 128d, 128n]
        o = o.transpose(0, 2, 1).reshape(NPC_PAD, D)
        out[c * NPC : (c + 1) * NPC] = o[:NPC]
    return out

